# revision 1
# baseline (speedup 1.0000x reference)
"""CantorAttention Trainium2 kernel.

Problem (hardcoded): B=2, S=2048, DIM=512, H=8 heads, D=64, K=64 routes.
  qkv = x @ w_qkv + b_qkv ; per-head sparse attention over routes[q, :] ;
  out = attn_out @ w_out + b_out.

Strategy (8 cores): shard batch x head-pairs. Core i handles batch i//4 and
heads (2*(i%4), 2*(i%4)+1). Routes are shared across batch/heads, so the
sparse attention is run DENSE on the TensorEngine with a host-precomputed
multiplicative count-mask C^T[k, q] = #{j : routes[q, j] == k}:

  P[k, q]  = C^T[k, q] * exp(scale * (K q_vec . k_vec))       (0 off-route)
  out_h    = (V^T_aug @ P) / denom,  denom from an appended ones-column
  partial  = concat_h(out_h) @ w_out[head rows]               (per core)

Host gathers: final[b] = sum of the 4 partials of batch b + b_out.
Exactly reproduces softmax over the 64 routed scores (duplicates included
via the count mask).

Everything on PE is bf16 with fp32 PSUM accumulation; exp on ScalarE;
mask-multiply on VectorE (bf16 2x mode); transposed layouts throughout so
no on-chip transposes are needed except V (one PE transpose per key tile).
"""

import numpy as np
import ml_dtypes

import concourse.bass as bass
import concourse.bacc as bacc
import concourse.mybir as mybir
import concourse.tile as tile
from concourse.bass_utils import run_bass_kernel_spmd
from concourse.masks import make_identity

BF16 = mybir.dt.bfloat16
F32 = mybir.dt.float32
NPBF16 = ml_dtypes.bfloat16

B = 2
S = 2048
DIM = 512
H = 8
D = 64
KR = 64
SCALE = 0.125

P = 128
NKT = S // P      # 16 key tiles
QC = 512          # query chunk (psum bank width)
NQC = S // QC     # 4 query chunks
NC4 = DIM // P    # 4 contraction chunks

_CACHE = {}


def build_nc():
    if "nc" in _CACHE:
        return _CACHE["nc"]
    nc = bacc.Bacc(
        "TRN2",
        target_bir_lowering=False,
        debug=False,
        num_devices=8,
    )

    xt_d = nc.dram_tensor("xt", [P, NC4 * S], BF16, kind="ExternalInput").ap()
    wq_d = nc.dram_tensor("wq", [P, NC4 * P], BF16, kind="ExternalInput").ap()
    wk_d = nc.dram_tensor("wk", [P, NC4 * P], BF16, kind="ExternalInput").ap()
    wv_d = nc.dram_tensor("wv", [P, NC4 * P], BF16, kind="ExternalInput").ap()
    bq_d = nc.dram_tensor("bq", [P, 1], F32, kind="ExternalInput").ap()
    bk_d = nc.dram_tensor("bk", [P, 1], F32, kind="ExternalInput").ap()
    bv_d = nc.dram_tensor("bv", [P, 1], F32, kind="ExternalInput").ap()
    ct_d = nc.dram_tensor("ct", [P, NKT * S], BF16, kind="ExternalInput").ap()
    wo_d = nc.dram_tensor("wo", [P, DIM], BF16, kind="ExternalInput").ap()
    out_d = nc.dram_tensor("out", [S, DIM], F32, kind="ExternalOutput").ap()

    with tile.TileContext(nc) as tc:
        with tc.tile_pool(name="persist", bufs=1) as pp:
            ident = pp.tile([P, P], BF16, tag="ident")
            make_identity(nc, ident[:])

            xt_big = pp.tile([P, NC4 * S], BF16, tag="xtb", name="xt_big")
            nc.sync.dma_start(out=xt_big[:], in_=xt_d[:, :])
            xt_sb = [xt_big[:, c * S:(c + 1) * S] for c in range(NC4)]

            w_sb = {}
            for name, wd in (("q", wq_d), ("k", wk_d), ("v", wv_d)):
                wt = pp.tile([P, NC4 * P], BF16, tag=f"w{name}b", name=f"w{name}_big")
                nc.sync.dma_start(out=wt[:], in_=wd[:, :])
                for c in range(NC4):
                    w_sb[(name, c)] = wt[:, c * P:(c + 1) * P]
            b_sb = {}
            for name, bd in (("q", bq_d), ("k", bk_d), ("v", bv_d)):
                t = pp.tile([P, 1], F32, tag=f"b{name}", name=f"b{name}_sb")
                nc.sync.dma_start(out=t[:], in_=bd[:, :])
                b_sb[name] = t

            wo_sb = pp.tile([P, DIM], BF16, tag="wo")
            nc.sync.dma_start(out=wo_sb[:], in_=wo_d[:, :])
            sel_sb = {}
            for h in range(2):
                t = pp.tile([P, P], F32, tag=f"sel{h}", name=f"sel{h}")
                nc.vector.memset(t[:], 0.0)
                nc.vector.memset(t[0:1, h * D:(h + 1) * D], 1.0)
                sel_sb[h] = t

            ct_big = pp.tile([P, NKT * S], BF16, tag="ctb", name="ct_big")
            nc.sync.dma_start(out=ct_big[:], in_=ct_d[:, :])
            ct_sb = [ct_big[:, kt * S:(kt + 1) * S] for kt in range(NKT)]

            # v^T stacked (2 heads); q^T/k^T per-head, rows 64-127 zero-padded
            # so every main-loop matmul is a full [128,128] stationary operand.
            qkvt = {}
            qkvt["v"] = pp.tile([P, S], BF16, tag="vt", name="vt")
            for name in ("q", "k"):
                for h in range(2):
                    t = pp.tile([P, S], BF16, tag=f"{name}t{h}", name=f"{name}t{h}")
                    nc.vector.memset(t[D:P, :], 0.0)
                    qkvt[(name, h)] = t

            # Phase 1: QKV^T = W^T @ X^T (+bias), bf16.
            with tc.tile_pool(name="psum_pre", bufs=4, space="PSUM") as pre:
                for name in ("k", "q", "v"):
                    for qc in range(NQC):
                        ps = pre.tile([P, QC], F32, tag="qkvps", name="qkvps")
                        for c in range(NC4):
                            nc.tensor.matmul(
                                ps[:],
                                lhsT=w_sb[(name, c)],
                                rhs=xt_sb[c][:, qc * QC:(qc + 1) * QC],
                                start=(c == 0),
                                stop=(c == NC4 - 1),
                            )
                        if name == "v":
                            nc.vector.tensor_tensor(
                                out=qkvt["v"][:, qc * QC:(qc + 1) * QC],
                                in0=ps[:],
                                in1=b_sb["v"][:].to_broadcast([P, QC]),
                                op=mybir.AluOpType.add,
                            )
                        else:
                            for h in range(2):
                                hd = h * D
                                nc.vector.tensor_tensor(
                                    out=qkvt[(name, h)][0:D, qc * QC:(qc + 1) * QC],
                                    in0=ps[hd:hd + D, :],
                                    in1=b_sb[name][hd:hd + D, :].to_broadcast([D, QC]),
                                    op=mybir.AluOpType.add,
                                )

                # Phase 1b: V tiles in [key, d] layout with ones column.
                v_sb = {}
                for h in range(2):
                    for kt in range(NKT):
                        v_sb[(h, kt)] = pp.tile([P, P], BF16, tag=f"v{h}_{kt}", name=f"v{h}_{kt}")
                for kt in range(NKT):
                    tp = pre.tile([P, P], BF16, tag="vtps", name="vtps")
                    nc.tensor.transpose(
                        out=tp[:], in_=qkvt["v"][:, kt * P:(kt + 1) * P],
                        identity=ident[:],
                    )
                    for h in range(2):
                        nc.scalar.copy(
                            out=v_sb[(h, kt)][:, 0:D], in_=tp[:, h * D:(h + 1) * D]
                        )
                        nc.vector.memset(v_sb[(h, kt)][:, D:D + 1], 1.0)
                        nc.vector.memset(v_sb[(h, kt)][:, D + 1:P], 0.0)

            ot_sb = pp.tile([P, S], F32, tag="ot")
            den_sb = {}
            for h in range(2):
                den_sb[h] = pp.tile([P, S], F32, tag=f"den{h}", name=f"den{h}")
                nc.vector.memset(den_sb[h][D:P, :], 0.0)
                nc.vector.memset(den_sb[h][0:D, :], 0.0)
            r2r_sb = pp.tile([P, S], F32, tag="r2r")
            on_sb = pp.tile([P, S], BF16, tag="on")

            # Phase 2: dense masked attention, one head at a time.
            QH = 1024
            for h in range(2):
                hd = h * D
                with tc.tile_pool(name=f"psum_s{h}", bufs=2, space="PSUM") as sp, \
                     tc.tile_pool(name=f"psum_ot{h}", bufs=1, space="PSUM") as op, \
                     tc.tile_pool(name=f"pwork{h}", bufs=6) as pw:
                    ot_ps = op.tile([P, S], F32, tag="otps", name="otps")
                    for kt in range(NKT):
                        for q2 in range(S // QH):
                            s_ps = sp.tile([P, QH], F32, tag="s", name="s_ps")
                            for half in range(QH // QC):
                                off = q2 * QH + half * QC
                                nc.tensor.matmul(
                                    s_ps[:, half * QC:(half + 1) * QC],
                                    lhsT=qkvt[("k", h)][:, kt * P:(kt + 1) * P],
                                    rhs=qkvt[("q", h)][:, off:off + QC],
                                    start=True,
                                    stop=True,
                                )
                            p_sb = pw.tile([P, QH], BF16, tag="p", name="p_sb")
                            nc.scalar.activation(
                                p_sb[:], s_ps[:], mybir.ActivationFunctionType.Exp
                            )
                            pm_sb = pw.tile([P, QH], BF16, tag="pm", name="pm_sb")
                            nc.vector.tensor_tensor(
                                out=pm_sb[:],
                                in0=p_sb[:],
                                in1=ct_sb[kt][:, q2 * QH:(q2 + 1) * QH],
                                op=mybir.AluOpType.mult,
                            )
                            for half in range(QH // QC):
                                off = q2 * QH + half * QC
                                nc.tensor.matmul(
                                    ot_ps[:, off:off + QC],
                                    lhsT=v_sb[(h, kt)][:],
                                    rhs=pm_sb[:, half * QC:(half + 1) * QC],
                                    start=(kt == 0),
                                    stop=(kt == NKT - 1),
                                )
                    nc.scalar.copy(out=ot_sb[hd:hd + D, :], in_=ot_ps[0:D, :])
                    nc.vector.tensor_copy(out=den_sb[h][0:1, :], in_=ot_ps[D:D + 1, :])

            # Phase 3: normalize, project, store (pipelined per 512-chunk).
            with tc.tile_pool(name="psum_r2", bufs=2, space="PSUM") as rp, \
                 tc.tile_pool(name="psum_fin", bufs=3, space="PSUM") as fp, \
                 tc.tile_pool(name="fin_sb", bufs=4) as fsb:
                r2_list = []
                for qc in range(NQC):
                    qs = slice(qc * QC, (qc + 1) * QC)
                    r2_ps = rp.tile([P, QC], F32, tag="r2", name="r2_ps", bufs=4)
                    for h in range(2):
                        nc.tensor.matmul(
                            r2_ps[:],
                            lhsT=sel_sb[h][:],
                            rhs=den_sb[h][:, qs],
                            start=(h == 0),
                            stop=(h == 1),
                        )
                    r2_list.append(r2_ps)
                for qc in range(NQC):
                    qs = slice(qc * QC, (qc + 1) * QC)
                    nc.vector.reciprocal_approx_fast(out=r2r_sb[:, qs], in_=r2_list[qc][:])
                    nc.vector.tensor_tensor(
                        out=on_sb[:, qs], in0=ot_sb[:, qs], in1=r2r_sb[:, qs],
                        op=mybir.AluOpType.mult,
                    )
                    for qt in range(qc * NC4, (qc + 1) * NC4):
                        pr = fp.tile([P, DIM], F32, tag="pr", name="pr_ps")
                        nc.tensor.matmul(
                            pr[:],
                            lhsT=on_sb[:, qt * P:(qt + 1) * P],
                            rhs=wo_sb[:],
                            start=True,
                            stop=True,
                        )
                        o_sb = fsb.tile([P, DIM], F32, tag="osb", name="o_sb")
                        nc.scalar.copy(out=o_sb[:], in_=pr[:])
                        nc.sync.dma_start(
                            out=out_d[qt * P:(qt + 1) * P, :], in_=o_sb[:]
                        )

    nc.compile()
    _CACHE["nc"] = nc
    return nc


def make_in_maps(x, routes, w_qkv, b_qkv, w_out):
    x = np.asarray(x, np.float32)
    routes = np.asarray(routes)
    w_qkv = np.asarray(w_qkv, np.float32)
    b_qkv = np.asarray(b_qkv, np.float32)
    w_out = np.asarray(w_out, np.float32)

    C = np.zeros((S, S), np.float32)
    np.add.at(C, (np.arange(S)[:, None], routes), 1.0)

    def pack(a):
        # [n*128, X] -> [128, n*X]
        n = a.shape[0] // P
        return np.ascontiguousarray(
            a.reshape(n, P, a.shape[1]).transpose(1, 0, 2).reshape(P, -1))

    xt = [pack(np.ascontiguousarray(x[b].T)).astype(NPBF16) for b in range(B)]
    ctp = pack(np.ascontiguousarray(C.T)).astype(NPBF16)

    in_maps = []
    for core in range(8):
        b = core // 4
        hp = core % 4
        col = hp * P
        wq = pack(w_qkv[:, col:col + P] * SCALE).astype(NPBF16)
        wk = pack(w_qkv[:, DIM + col:DIM + col + P]).astype(NPBF16)
        wv = pack(w_qkv[:, 2 * DIM + col:2 * DIM + col + P]).astype(NPBF16)
        bq = (b_qkv[col:col + P] * SCALE).astype(np.float32).reshape(P, 1)
        bk = b_qkv[DIM + col:DIM + col + P].astype(np.float32).reshape(P, 1)
        bv = b_qkv[2 * DIM + col:2 * DIM + col + P].astype(np.float32).reshape(P, 1)
        wo = np.ascontiguousarray(w_out[col:col + P, :]).astype(NPBF16)
        in_maps.append(dict(
            xt=xt[b], wq=wq, wk=wk, wv=wv, bq=bq, bk=bk, bv=bv,
            ct=ctp, wo=wo,
        ))
    return in_maps


def run(inputs, trace=False, trace_cores=None):
    nc = build_nc()
    in_maps = make_in_maps(
        inputs["x"], inputs["routes"], inputs["w_qkv"], inputs["b_qkv"],
        inputs["w_out"],
    )
    res = run_bass_kernel_spmd(
        nc, in_maps, list(range(8)), trace=trace, trace_cores=trace_cores,
    )
    b_out = np.asarray(inputs["b_out"], np.float32)
    final = np.zeros((B, S, DIM), np.float32)
    for core in range(8):
        final[core // 4] += res.results[core]["out"]
    final += b_out[None, None, :]
    return final, res


def kernel(**inputs):
    final, _ = run(inputs, trace=False)
    return final



# revision 11
# speedup vs baseline: 1.3107x; 1.3107x over previous
"""CantorAttention Trainium2 kernel — banded formulation.

Problem (hardcoded): B=2, S=2048, DIM=512, H=8 heads, D=64, K=64 routes.
  qkv = x @ w_qkv + b_qkv ; per-head sparse attention over routes[q, :] ;
  out = attn_out @ w_out + b_out.

Strategy (8 cores): shard batch x head-pairs. Core i handles batch i//4 and
heads (2*(i%4), 2*(i%4)+1).

Key idea: routes come from k-nearest Cantor coordinates. Sorting queries and
keys by Cantor coordinate (a host-side permutation, free) makes the 0/1
route-count mask C banded: only ~46 of 256 [128x128] tiles are nonzero.
The tile schedule is derived at runtime from the actual `routes` input, so
the kernel stays correct for arbitrary routes (it just degrades to dense).

Per 128x128 band tile (k-tile kt vs query range):
  S[k, q]  = K^T q          (contraction 64, PE)
  P        = exp(S)         (ScalarE)
  Pm       = P * C^T        (VectorE, bf16)
  ot[d|den,q] += [V|1]^T Pm (PE, ones column accumulates the denominator)
then out = ((ot/den)^T stacked heads) @ w_out[head rows], partials summed on
host. All PE work in bf16 with fp32 PSUM; output partials in bf16.
"""

import numpy as np
import ml_dtypes

import concourse.bass as bass
import concourse.bacc as bacc
import concourse.mybir as mybir
import concourse.tile as tile
from concourse.bass_utils import run_bass_kernel_spmd

BF16 = mybir.dt.bfloat16
F32 = mybir.dt.float32
NPBF16 = ml_dtypes.bfloat16

B = 2
S = 2048
DIM = 512
H = 8
D = 64
KR = 64
SCALE = 0.125

P = 128
NT = S // P       # 16 tiles of 128 along seq
NC4 = DIM // P    # 4 contraction chunks

_CACHE = {}


def _cantor_perm():
    x = np.arange(S, dtype=np.float32) / max(1, S - 1)
    x = np.clip(x, 1e-6, 1.0 - 1e-6)
    val = np.zeros_like(x)
    factor = 0.5
    for _ in range(8):
        xs = x * 3.0
        digit = np.floor(xs).astype(np.int64)
        x = xs - digit.astype(np.float32)
        val = val + (digit == 2).astype(np.float32) * factor
        factor *= 0.5
    coords = np.clip(val, 0.0, 1.0)
    return np.lexsort((np.arange(S), coords))


def _schedule(routes):
    """Derive the banded tile schedule from the actual routes input.

    Returns (perm, entries, first_e, last_e, ct_packed) where entries is a
    tuple of (kt, qlo, ntiles) with ntiles<=4, and ct_packed is the bf16
    count-mask blocks [128, 128*sum(ntiles)] in entry order.
    """
    perm = _cantor_perm()
    C = np.zeros((S, S), np.float32)
    np.add.at(C, (np.arange(S)[:, None], np.asarray(routes)), 1.0)
    Cp = C[perm][:, perm]
    occ = Cp.reshape(NT, P, NT, P).any(axis=(1, 3))  # [qt, kt]

    entries = []
    for kt in range(NT):
        qts = np.nonzero(occ[:, kt])[0]
        if len(qts) == 0:
            continue
        lo, hi = int(qts.min()), int(qts.max()) + 1
        start = lo
        while start < hi:
            n = min(4, hi - start)
            entries.append((kt, start, n))
            start += n
    entries = tuple(entries)

    first_e = [None] * NT
    last_e = [None] * NT
    for e, (kt, qlo, n) in enumerate(entries):
        for qt in range(qlo, qlo + n):
            if first_e[qt] is None:
                first_e[qt] = e
            last_e[qt] = e
    assert all(f is not None for f in first_e), "query tile with no routes"

    blocks = []
    for (kt, qlo, n) in entries:
        blk = Cp[qlo * P:(qlo + n) * P, kt * P:(kt + 1) * P].T  # [128 k, W q]
        blocks.append(np.ascontiguousarray(blk))
    ct_packed = np.concatenate(blocks, axis=1).astype(NPBF16)
    return perm, entries, tuple(first_e), tuple(last_e), ct_packed


def build_nc(entries, first_e, last_e):
    key = ("nc", entries, first_e, last_e)
    if key in _CACHE:
        return _CACHE[key]
    nc = bacc.Bacc(
        "TRN2",
        target_bir_lowering=False,
        debug=False,
        num_devices=8,
    )

    ctw = P * sum(n for (_, _, n) in entries)

    wq_d = nc.dram_tensor("wq", [P, NC4 * P], BF16, kind="ExternalInput").ap()
    wk_d = nc.dram_tensor("wk", [P, NC4 * P], BF16, kind="ExternalInput").ap()
    wv_d = nc.dram_tensor("wv", [P, NC4 * P], BF16, kind="ExternalInput").ap()
    bq_d = nc.dram_tensor("bq", [P, 1], F32, kind="ExternalInput").ap()
    bk_d = nc.dram_tensor("bk", [P, 1], F32, kind="ExternalInput").ap()
    bv_d = nc.dram_tensor("bv", [P, 1], F32, kind="ExternalInput").ap()
    xt_d = {}
    for c in range(NC4):
        for g in range(2):
            xt_d[(c, g)] = nc.dram_tensor(
                f"xt_{c}_{g}", [P, 1024], BF16, kind="ExternalInput").ap()
    ct_d = nc.dram_tensor("ct", [P, ctw], BF16, kind="ExternalInput").ap()
    wo_d = nc.dram_tensor("wo", [P, DIM], BF16, kind="ExternalInput").ap()
    out_d = nc.dram_tensor("out", [P, 4 * S], BF16, kind="ExternalOutput").ap()

    with tile.TileContext(nc) as tc:
        with tc.tile_pool(name="persist", bufs=1) as pp:
            # --- persistent SBUF + input DMAs (sync queue, in order) ---
            w_sb = {}
            for name, wd in (("q", wq_d), ("k", wk_d), ("v", wv_d)):
                t = pp.tile([P, NC4 * P], BF16, tag=f"w{name}", name=f"w{name}_sb")
                nc.sync.dma_start(out=t[:], in_=wd[:, :])
                w_sb[name] = t
            b_sb = {}
            for name, bd in (("q", bq_d), ("k", bk_d), ("v", bv_d)):
                t = pp.tile([P, 1], F32, tag=f"b{name}", name=f"b{name}_sb")
                nc.sync.dma_start(out=t[:], in_=bd[:, :])
                b_sb[name] = t
            xt_sb = {}
            for g in range(2):
                for c in range(NC4):
                    t = pp.tile([P, 1024], BF16, tag=f"xt{c}{g}", name=f"xt{c}{g}")
                    nc.sync.dma_start(out=t[:], in_=xt_d[(c, g)][:, :])
                    xt_sb[(c, g)] = t
            ct_sb = pp.tile([P, ctw], BF16, tag="ct", name="ct_sb")
            nc.sync.dma_start(out=ct_sb[:], in_=ct_d[:, :])
            wo_sb = pp.tile([P, DIM], BF16, tag="wo", name="wo_sb")
            nc.sync.dma_start(out=wo_sb[:], in_=wo_d[:, :])

            qh = [pp.tile([D, S], BF16, tag=f"qh{h}", name=f"qh{h}") for h in range(2)]
            kh = [pp.tile([D, S], BF16, tag=f"kh{h}", name=f"kh{h}") for h in range(2)]
            vt = pp.tile([P, S], BF16, tag="vt", name="vt")
            # [V_h | ones] per (head, key tile): lhsT for PV, row 64 = denom
            v2 = {}
            for h in range(2):
                for kt in range(NT):
                    v2[(h, kt)] = pp.tile([P, D + 1], BF16, tag=f"v2_{h}_{kt}",
                                          name=f"v2_{h}_{kt}")
            ot_sb = pp.tile([P, S], BF16, tag="ot", name="ot_sb")
            den_t = [pp.tile([1, S], BF16, tag=f"den{h}", name=f"den{h}")
                     for h in range(2)]
            sel_t = [pp.tile([1, P], BF16, tag=f"sel{h}", name=f"sel{h}")
                     for h in range(2)]
            nc.vector.memset(sel_t[0][0:1, 0:D], 1.0)
            nc.vector.memset(sel_t[0][0:1, D:P], 0.0)
            nc.vector.memset(sel_t[1][0:1, 0:D], 0.0)
            nc.vector.memset(sel_t[1][0:1, D:P], 1.0)
            r2r = pp.tile([P, S], F32, tag="r2r", name="r2r")
            on_sb = pp.tile([P, S], BF16, tag="on", name="on_sb")
            for h in range(2):
                for kt in range(NT):
                    nc.gpsimd.memset(v2[(h, kt)][:, D:D + 1], 1.0)

            # --- phase 1: QKV^T = W^T @ X^T, bias folded into the copies ---
            with tc.tile_pool(name="ps1", bufs=2, space="PSUM") as p1:
                for qc in range(4):
                    g, half = qc // 2, qc % 2
                    ps = {}
                    for name in ("q", "k", "v"):
                        ps[name] = p1.tile([P, 512], F32, tag=f"p1{name}",
                                           name=f"p1{name}")
                        for c in range(NC4):
                            nc.tensor.matmul(
                                ps[name][:],
                                lhsT=w_sb[name][:, c * P:(c + 1) * P],
                                rhs=xt_sb[(c, g)][:, half * 512:(half + 1) * 512],
                                start=(c == 0),
                                stop=(c == NC4 - 1),
                            )
                    qs = slice(qc * 512, (qc + 1) * 512)
                    for name, dest in (("q", qh), ("k", kh)):
                        for h in range(2):
                            nc.scalar.activation(
                                dest[h][:, qs], ps[name][h * D:(h + 1) * D, :],
                                mybir.ActivationFunctionType.Identity,
                                bias=b_sb[name][h * D:(h + 1) * D, :],
                            )
                    nc.scalar.activation(
                        vt[:, qs], ps["v"][:],
                        mybir.ActivationFunctionType.Identity,
                        bias=b_sb["v"][:],
                    )

            # --- phase 1b: V into [key, d] layout via DMA transpose ---
            for kt in range(NT):
                ks = slice(kt * P, (kt + 1) * P)
                nc.sync.dma_start_transpose(out=v2[(0, kt)][:, 0:D], in_=vt[0:D, ks])
                nc.sync.dma_start_transpose(out=v2[(1, kt)][:, 0:D], in_=vt[D:P, ks])

            # --- phase 2: banded masked attention, one head at a time ---
            with tc.tile_pool(name="sp", bufs=3, space="PSUM") as sp, \
                 tc.tile_pool(name="pw", bufs=4) as pw:
                for h in range(2):
                    with tc.tile_pool(name=f"op{h}", bufs=1, space="PSUM") as op:
                        ot_ps = op.tile([P, S], F32, tag="otps", name="otps")
                        off = 0
                        for e, (kt, qlo, n) in enumerate(entries):
                            W = n * P
                            s_ps = sp.tile([P, 512], F32, tag="s", name="s_ps")
                            nc.tensor.matmul(
                                s_ps[:, 0:W],
                                lhsT=kh[h][:, kt * P:(kt + 1) * P],
                                rhs=qh[h][:, qlo * P:qlo * P + W],
                                start=True, stop=True,
                            )
                            p_sb = pw.tile([P, 512], BF16, tag="p", name="p_sb")
                            nc.scalar.activation(
                                p_sb[:, 0:W], s_ps[:, 0:W],
                                mybir.ActivationFunctionType.Exp,
                            )
                            pm_sb = pw.tile([P, 512], BF16, tag="pm", name="pm_sb")
                            nc.vector.tensor_tensor(
                                out=pm_sb[:, 0:W], in0=p_sb[:, 0:W],
                                in1=ct_sb[:, off:off + W],
                                op=mybir.AluOpType.mult,
                            )
                            for j in range(n):
                                qt = qlo + j
                                # interleave qt -> bank qt%4 so concurrently
                                # open accumulation groups use distinct banks
                                pc = (qt % 4) * 512 + (qt // 4) * P
                                nc.tensor.matmul(
                                    ot_ps[0:D + 1, pc:pc + P],
                                    lhsT=v2[(h, kt)][:],
                                    rhs=pm_sb[:, j * P:(j + 1) * P],
                                    start=(e == first_e[qt]),
                                    stop=(e == last_e[qt]),
                                )
                            off += W
                        # drain: rows 0-63 -> ot, row 64 -> den (split engines)
                        hs = slice(h * D, (h + 1) * D)
                        nc.vector.tensor_copy(
                            out=ot_sb[hs, 0:1024], in_=ot_ps[0:D, 0:1024])
                        nc.vector.tensor_copy(
                            out=ot_sb[hs, 1024:S], in_=ot_ps[0:D, 1024:S])
                        nc.vector.tensor_copy(
                            out=den_t[h][0:1, :], in_=ot_ps[D:D + 1, :])

            # --- phase 3: normalize + output projection (transposed out) ---
            with tc.tile_pool(name="rp", bufs=2, space="PSUM") as rp:
                for qc in range(4):
                    qs = slice(qc * 512, (qc + 1) * 512)
                    r2_ps = rp.tile([P, 512], F32, tag="r2", name="r2_ps")
                    for h in range(2):
                        nc.tensor.matmul(
                            r2_ps[:], lhsT=sel_t[h][:], rhs=den_t[h][:, qs],
                            start=(h == 0), stop=(h == 1),
                        )
                    nc.vector.reciprocal_approx_fast(out=r2r[:, qs], in_=r2_ps[:])
                    nc.vector.tensor_tensor(
                        out=on_sb[:, qs], in0=ot_sb[:, qs], in1=r2r[:, qs],
                        op=mybir.AluOpType.mult,
                    )
            with tc.tile_pool(name="fp", bufs=3, space="PSUM") as fp, \
                 tc.tile_pool(name="fsb", bufs=3) as fsb:
                for oc in range(4):
                    for g in range(2):
                        f_ps = fp.tile([P, 1024], F32, tag="f", name="f_ps")
                        for qq in range(2):
                            cs = slice(g * 1024 + qq * 512,
                                       g * 1024 + (qq + 1) * 512)
                            nc.tensor.matmul(
                                f_ps[:, qq * 512:(qq + 1) * 512],
                                lhsT=wo_sb[:, oc * P:(oc + 1) * P],
                                rhs=on_sb[:, cs],
                                start=True, stop=True,
                            )
                        f_sb = fsb.tile([P, 1024], BF16, tag="fsb", name="f_sb")
                        if g == 0:
                            nc.vector.tensor_copy(out=f_sb[:], in_=f_ps[:])
                        else:
                            nc.scalar.copy(out=f_sb[:], in_=f_ps[:])
                        nc.scalar.dma_start(
                            out=out_d[:, oc * S + g * 1024:oc * S + (g + 1) * 1024],
                            in_=f_sb[:],
                        )

    nc.compile()
    _CACHE[key] = nc
    return nc


def _pack_w(a):
    # [512, 128] -> [128, 4*128] with row chunk c at cols c*128..
    return np.ascontiguousarray(
        a.reshape(NC4, P, P).transpose(1, 0, 2).reshape(P, NC4 * P))


def make_in_maps(x, routes, w_qkv, b_qkv, w_out):
    x = np.asarray(x, np.float32)
    w_qkv = np.asarray(w_qkv, np.float32)
    b_qkv = np.asarray(b_qkv, np.float32)
    w_out = np.asarray(w_out, np.float32)

    perm, entries, first_e, last_e, ct_packed = _schedule(routes)

    xts = {}
    for b in range(B):
        xT = np.ascontiguousarray(x[b][perm].T)  # [512, 2048] permuted queries
        for c in range(NC4):
            for g in range(2):
                xts[(b, c, g)] = np.ascontiguousarray(
                    xT[c * P:(c + 1) * P, g * 1024:(g + 1) * 1024]).astype(NPBF16)

    in_maps = []
    for core in range(8):
        b = core // 4
        hp = core % 4
        col = hp * P
        m = dict(
            wq=_pack_w(w_qkv[:, col:col + P] * SCALE).astype(NPBF16),
            wk=_pack_w(w_qkv[:, DIM + col:DIM + col + P]).astype(NPBF16),
            wv=_pack_w(w_qkv[:, 2 * DIM + col:2 * DIM + col + P]).astype(NPBF16),
            bq=(b_qkv[col:col + P] * SCALE).astype(np.float32).reshape(P, 1),
            bk=b_qkv[DIM + col:DIM + col + P].astype(np.float32).reshape(P, 1),
            bv=b_qkv[2 * DIM + col:2 * DIM + col + P].astype(np.float32).reshape(P, 1),
            ct=ct_packed,
            wo=np.ascontiguousarray(w_out[col:col + P, :]).astype(NPBF16),
        )
        for c in range(NC4):
            for g in range(2):
                m[f"xt_{c}_{g}"] = xts[(b, c, g)]
        in_maps.append(m)
    return in_maps, perm, entries, first_e, last_e


_COLMAP = np.array([(q // P % 4) * 512 + (q // P // 4) * P + q % P
                    for q in range(S)])


def unpack_out(arr, perm):
    """[128, 4*2048] bf16 core output -> [2048, 512] f32 in original order."""
    outT = np.zeros((DIM, S), np.float32)
    a = np.asarray(arr, np.float32)
    for oc in range(4):
        outT[oc * P:(oc + 1) * P, :] = a[:, oc * S:(oc + 1) * S]
    out_perm = outT[:, _COLMAP].T  # [2048, 512], rows are permuted queries
    out = np.zeros((S, DIM), np.float32)
    out[perm] = out_perm
    return out


def run(inputs, trace=False, trace_cores=None):
    in_maps, perm, entries, first_e, last_e = make_in_maps(
        inputs["x"], inputs["routes"], inputs["w_qkv"], inputs["b_qkv"],
        inputs["w_out"],
    )
    nc = build_nc(entries, first_e, last_e)
    res = run_bass_kernel_spmd(
        nc, in_maps, list(range(8)), trace=trace, trace_cores=trace_cores,
    )
    b_out = np.asarray(inputs["b_out"], np.float32)
    final = np.zeros((B, S, DIM), np.float32)
    for core in range(8):
        final[core // 4] += unpack_out(res.results[core]["out"], perm)
    final += b_out[None, None, :]
    return final, res


def kernel(**inputs):
    final, _ = run(inputs, trace=False)
    return final


# revision 14
# speedup vs baseline: 1.5207x; 1.1602x over previous
"""CantorAttention Trainium2 kernel — banded formulation.

Problem (hardcoded): B=2, S=2048, DIM=512, H=8 heads, D=64, K=64 routes.
  qkv = x @ w_qkv + b_qkv ; per-head sparse attention over routes[q, :] ;
  out = attn_out @ w_out + b_out.

Strategy (8 cores): shard batch x head-pairs. Core i handles batch i//4 and
heads (2*(i%4), 2*(i%4)+1).

Key idea: routes come from k-nearest Cantor coordinates. Sorting queries and
keys by Cantor coordinate (a host-side permutation, free) makes the 0/1
route-count mask C banded: only ~46 of 256 [128x128] tiles are nonzero.
The tile schedule is derived at runtime from the actual `routes` input, so
the kernel stays correct for arbitrary routes (it just degrades to dense).

Per 128x128 band tile (k-tile kt vs query range):
  S[k, q]  = K^T q          (contraction 64, PE)
  P        = exp(S)         (ScalarE)
  Pm       = P * C^T        (VectorE, bf16)
  ot[d|den,q] += [V|1]^T Pm (PE, ones column accumulates the denominator)
then out = ((ot/den)^T stacked heads) @ w_out[head rows], partials summed on
host. All PE work in bf16 with fp32 PSUM; output partials in bf16.
"""

import numpy as np
import ml_dtypes

import concourse.bass as bass
import concourse.bacc as bacc
import concourse.mybir as mybir
import concourse.tile as tile
from concourse.bass_utils import run_bass_kernel_spmd
from concourse.masks import make_identity

BF16 = mybir.dt.bfloat16
F32 = mybir.dt.float32
NPBF16 = ml_dtypes.bfloat16

B = 2
S = 2048
DIM = 512
H = 8
D = 64
KR = 64
SCALE = 0.125

P = 128
NT = S // P       # 16 tiles of 128 along seq
NC4 = DIM // P    # 4 contraction chunks

_CACHE = {}


def _cantor_perm():
    x = np.arange(S, dtype=np.float32) / max(1, S - 1)
    x = np.clip(x, 1e-6, 1.0 - 1e-6)
    val = np.zeros_like(x)
    factor = 0.5
    for _ in range(8):
        xs = x * 3.0
        digit = np.floor(xs).astype(np.int64)
        x = xs - digit.astype(np.float32)
        val = val + (digit == 2).astype(np.float32) * factor
        factor *= 0.5
    coords = np.clip(val, 0.0, 1.0)
    return np.lexsort((np.arange(S), coords))


def _schedule(routes):
    """Derive the banded tile schedule from the actual routes input.

    Returns (perm, entries, first_e, last_e, ct_packed) where entries is a
    tuple of (kt, qlo, ntiles) with ntiles<=4, and ct_packed is the bf16
    count-mask blocks [128, 128*sum(ntiles)] in entry order.
    """
    perm = _cantor_perm()
    C = np.zeros((S, S), np.float32)
    np.add.at(C, (np.arange(S)[:, None], np.asarray(routes)), 1.0)
    Cp = C[perm][:, perm]
    occ = Cp.reshape(NT, P, NT, P).any(axis=(1, 3))  # [qt, kt]

    entries = []
    for kt in range(NT):
        qts = np.nonzero(occ[:, kt])[0]
        if len(qts) == 0:
            continue
        lo, hi = int(qts.min()), int(qts.max()) + 1
        start = lo
        while start < hi:
            n = min(4, hi - start)
            entries.append((kt, start, n))
            start += n
    entries = tuple(entries)

    first_e = [None] * NT
    last_e = [None] * NT
    for e, (kt, qlo, n) in enumerate(entries):
        for qt in range(qlo, qlo + n):
            if first_e[qt] is None:
                first_e[qt] = e
            last_e[qt] = e
    assert all(f is not None for f in first_e), "query tile with no routes"

    blocks = []
    for (kt, qlo, n) in entries:
        blk = Cp[qlo * P:(qlo + n) * P, kt * P:(kt + 1) * P].T  # [128 k, W q]
        blocks.append(np.ascontiguousarray(blk))
    ct_packed = np.concatenate(blocks, axis=1).astype(NPBF16)
    return perm, entries, tuple(first_e), tuple(last_e), ct_packed


def build_nc(entries, first_e, last_e):
    key = ("nc", entries, first_e, last_e)
    if key in _CACHE:
        return _CACHE[key]
    nc = bacc.Bacc(
        "TRN2",
        target_bir_lowering=False,
        debug=False,
        num_devices=8,
    )

    ctw = P * sum(n for (_, _, n) in entries)

    wq_d = nc.dram_tensor("wq", [P, NC4 * P], BF16, kind="ExternalInput").ap()
    wk_d = nc.dram_tensor("wk", [P, NC4 * P], BF16, kind="ExternalInput").ap()
    wv_d = nc.dram_tensor("wv", [P, NC4 * P], BF16, kind="ExternalInput").ap()
    bq_d = nc.dram_tensor("bq", [P, 1], F32, kind="ExternalInput").ap()
    bk_d = nc.dram_tensor("bk", [P, 1], F32, kind="ExternalInput").ap()
    bv_d = nc.dram_tensor("bv", [P, 1], F32, kind="ExternalInput").ap()
    xt_d = {}
    for c in range(NC4):
        for g in range(2):
            xt_d[(c, g)] = nc.dram_tensor(
                f"xt_{c}_{g}", [P, 1024], BF16, kind="ExternalInput").ap()
    ct_d = nc.dram_tensor("ct", [P, ctw], BF16, kind="ExternalInput").ap()
    wo_d = nc.dram_tensor("wo", [P, DIM], BF16, kind="ExternalInput").ap()
    out_d = nc.dram_tensor("out", [P, 4 * S], BF16, kind="ExternalOutput").ap()

    with tile.TileContext(nc) as tc:
        with tc.tile_pool(name="persist", bufs=1) as pp:
            # --- persistent SBUF + input DMAs (sync queue, in order) ---
            w_sb = {}
            for name, wd in (("q", wq_d), ("k", wk_d), ("v", wv_d)):
                t = pp.tile([P, NC4 * P], BF16, tag=f"w{name}", name=f"w{name}_sb")
                nc.sync.dma_start(out=t[:], in_=wd[:, :])
                w_sb[name] = t
            b_sb = {}
            for name, bd in (("q", bq_d), ("k", bk_d), ("v", bv_d)):
                t = pp.tile([P, 1], F32, tag=f"b{name}", name=f"b{name}_sb")
                nc.sync.dma_start(out=t[:], in_=bd[:, :])
                b_sb[name] = t
            xt_sb = {}
            for g in range(2):
                for c in range(NC4):
                    t = pp.tile([P, 1024], BF16, tag=f"xt{c}{g}", name=f"xt{c}{g}")
                    nc.sync.dma_start(out=t[:], in_=xt_d[(c, g)][:, :])
                    xt_sb[(c, g)] = t
            ct_sb = pp.tile([P, ctw], BF16, tag="ct", name="ct_sb")
            nc.sync.dma_start(out=ct_sb[:], in_=ct_d[:, :])
            wo_sb = pp.tile([P, DIM], BF16, tag="wo", name="wo_sb")
            nc.sync.dma_start(out=wo_sb[:], in_=wo_d[:, :])

            qh = [pp.tile([D, S], BF16, tag=f"qh{h}", name=f"qh{h}") for h in range(2)]
            kh = [pp.tile([D, S], BF16, tag=f"kh{h}", name=f"kh{h}") for h in range(2)]
            vt = pp.tile([P, S], BF16, tag="vt", name="vt")
            # [V_h | ones] per (head, key tile): lhsT for PV, row 64 = denom
            v2 = {}
            for h in range(2):
                for kt in range(NT):
                    v2[(h, kt)] = pp.tile([P, D + 1], BF16, tag=f"v2_{h}_{kt}",
                                          name=f"v2_{h}_{kt}")
            ot_sb = pp.tile([P, S], BF16, tag="ot", name="ot_sb")
            den_t = [pp.tile([1, S], BF16, tag=f"den{h}", name=f"den{h}")
                     for h in range(2)]
            sel_t = [pp.tile([1, P], BF16, tag=f"sel{h}", name=f"sel{h}")
                     for h in range(2)]
            nc.vector.memset(sel_t[0][0:1, 0:D], 1.0)
            nc.vector.memset(sel_t[0][0:1, D:P], 0.0)
            nc.vector.memset(sel_t[1][0:1, 0:D], 0.0)
            nc.vector.memset(sel_t[1][0:1, D:P], 1.0)
            r2r = pp.tile([P, S], F32, tag="r2r", name="r2r")
            on_sb = pp.tile([P, S], BF16, tag="on", name="on_sb")
            ident = pp.tile([P, P], BF16, tag="ident")
            make_identity(nc, ident[:])
            for h in range(2):
                for kt in range(NT):
                    nc.gpsimd.memset(v2[(h, kt)][:, D:D + 1], 1.0)

            # --- phase 1: QKV^T = W^T @ X^T, bias folded into the copies ---
            with tc.tile_pool(name="ps1", bufs=2, space="PSUM") as p1:
                for qc in range(4):
                    g, half = qc // 2, qc % 2
                    ps = {}
                    for name in ("q", "k", "v"):
                        ps[name] = p1.tile([P, 512], F32, tag=f"p1{name}",
                                           name=f"p1{name}")
                        for c in range(NC4):
                            nc.tensor.matmul(
                                ps[name][:],
                                lhsT=w_sb[name][:, c * P:(c + 1) * P],
                                rhs=xt_sb[(c, g)][:, half * 512:(half + 1) * 512],
                                start=(c == 0),
                                stop=(c == NC4 - 1),
                            )
                    qs = slice(qc * 512, (qc + 1) * 512)
                    for name, dest in (("q", qh), ("k", kh)):
                        for h in range(2):
                            nc.scalar.activation(
                                dest[h][:, qs], ps[name][h * D:(h + 1) * D, :],
                                mybir.ActivationFunctionType.Identity,
                                bias=b_sb[name][h * D:(h + 1) * D, :],
                            )
                    nc.scalar.activation(
                        vt[:, qs], ps["v"][:],
                        mybir.ActivationFunctionType.Identity,
                        bias=b_sb["v"][:],
                    )

            # --- phase 1b: V into [key, d] layout via PE transpose ---
            with tc.tile_pool(name="tp", bufs=2, space="PSUM") as tpp:
                for kt in range(NT):
                    ks = slice(kt * P, (kt + 1) * P)
                    tp = tpp.tile([P, P], BF16, tag="tp", name="tp")
                    nc.tensor.transpose(out=tp[:], in_=vt[:, ks], identity=ident[:])
                    nc.vector.tensor_copy(out=v2[(0, kt)][:, 0:D], in_=tp[:, 0:D])
                    nc.vector.tensor_copy(out=v2[(1, kt)][:, 0:D], in_=tp[:, D:P])

            # --- phase 2: banded masked attention, one head at a time ---
            with tc.tile_pool(name="sp", bufs=3, space="PSUM") as sp, \
                 tc.tile_pool(name="pw", bufs=4) as pw:
                for h in range(2):
                    with tc.tile_pool(name=f"op{h}", bufs=1, space="PSUM") as op:
                        ot_ps = op.tile([P, S], F32, tag="otps", name="otps")
                        off = 0
                        for e, (kt, qlo, n) in enumerate(entries):
                            W = n * P
                            s_ps = sp.tile([P, 512], F32, tag="s", name="s_ps")
                            nc.tensor.matmul(
                                s_ps[:, 0:W],
                                lhsT=kh[h][:, kt * P:(kt + 1) * P],
                                rhs=qh[h][:, qlo * P:qlo * P + W],
                                start=True, stop=True,
                            )
                            p_sb = pw.tile([P, 512], BF16, tag="p", name="p_sb")
                            nc.scalar.activation(
                                p_sb[:, 0:W], s_ps[:, 0:W],
                                mybir.ActivationFunctionType.Exp,
                            )
                            pm_sb = pw.tile([P, 512], BF16, tag="pm", name="pm_sb")
                            nc.vector.tensor_tensor(
                                out=pm_sb[:, 0:W], in0=p_sb[:, 0:W],
                                in1=ct_sb[:, off:off + W],
                                op=mybir.AluOpType.mult,
                            )
                            for j in range(n):
                                qt = qlo + j
                                # interleave qt -> bank qt%4 so concurrently
                                # open accumulation groups use distinct banks
                                pc = (qt % 4) * 512 + (qt // 4) * P
                                nc.tensor.matmul(
                                    ot_ps[0:D + 1, pc:pc + P],
                                    lhsT=v2[(h, kt)][:],
                                    rhs=pm_sb[:, j * P:(j + 1) * P],
                                    start=(e == first_e[qt]),
                                    stop=(e == last_e[qt]),
                                )
                            off += W
                        # drain: rows 0-63 -> ot, row 64 -> den (split engines)
                        hs = slice(h * D, (h + 1) * D)
                        nc.vector.tensor_copy(
                            out=ot_sb[hs, 0:1024], in_=ot_ps[0:D, 0:1024])
                        nc.vector.tensor_copy(
                            out=ot_sb[hs, 1024:S], in_=ot_ps[0:D, 1024:S])
                        nc.vector.tensor_copy(
                            out=den_t[h][0:1, :], in_=ot_ps[D:D + 1, :])

            # --- phase 3: normalize + output projection (transposed out) ---
            with tc.tile_pool(name="rp", bufs=2, space="PSUM") as rp:
                for qc in range(4):
                    qs = slice(qc * 512, (qc + 1) * 512)
                    r2_ps = rp.tile([P, 512], F32, tag="r2", name="r2_ps")
                    for h in range(2):
                        nc.tensor.matmul(
                            r2_ps[:], lhsT=sel_t[h][:], rhs=den_t[h][:, qs],
                            start=(h == 0), stop=(h == 1),
                        )
                    nc.vector.reciprocal_approx_fast(out=r2r[:, qs], in_=r2_ps[:])
                    nc.vector.tensor_tensor(
                        out=on_sb[:, qs], in0=ot_sb[:, qs], in1=r2r[:, qs],
                        op=mybir.AluOpType.mult,
                    )
            with tc.tile_pool(name="fp", bufs=3, space="PSUM") as fp, \
                 tc.tile_pool(name="fsb", bufs=3) as fsb:
                for oc in range(4):
                    for g in range(2):
                        f_ps = fp.tile([P, 1024], F32, tag="f", name="f_ps")
                        for qq in range(2):
                            cs = slice(g * 1024 + qq * 512,
                                       g * 1024 + (qq + 1) * 512)
                            nc.tensor.matmul(
                                f_ps[:, qq * 512:(qq + 1) * 512],
                                lhsT=wo_sb[:, oc * P:(oc + 1) * P],
                                rhs=on_sb[:, cs],
                                start=True, stop=True,
                            )
                        f_sb = fsb.tile([P, 1024], BF16, tag="fsb", name="f_sb")
                        if g == 0:
                            nc.vector.tensor_copy(out=f_sb[:], in_=f_ps[:])
                        else:
                            nc.scalar.copy(out=f_sb[:], in_=f_ps[:])
                        nc.scalar.dma_start(
                            out=out_d[:, oc * S + g * 1024:oc * S + (g + 1) * 1024],
                            in_=f_sb[:],
                        )

    nc.compile()
    _CACHE[key] = nc
    return nc


def _pack_w(a):
    # [512, 128] -> [128, 4*128] with row chunk c at cols c*128..
    return np.ascontiguousarray(
        a.reshape(NC4, P, P).transpose(1, 0, 2).reshape(P, NC4 * P))


def make_in_maps(x, routes, w_qkv, b_qkv, w_out):
    x = np.asarray(x, np.float32)
    w_qkv = np.asarray(w_qkv, np.float32)
    b_qkv = np.asarray(b_qkv, np.float32)
    w_out = np.asarray(w_out, np.float32)

    perm, entries, first_e, last_e, ct_packed = _schedule(routes)

    xts = {}
    for b in range(B):
        xT = np.ascontiguousarray(x[b][perm].T)  # [512, 2048] permuted queries
        for c in range(NC4):
            for g in range(2):
                xts[(b, c, g)] = np.ascontiguousarray(
                    xT[c * P:(c + 1) * P, g * 1024:(g + 1) * 1024]).astype(NPBF16)

    in_maps = []
    for core in range(8):
        b = core // 4
        hp = core % 4
        col = hp * P
        m = dict(
            wq=_pack_w(w_qkv[:, col:col + P] * SCALE).astype(NPBF16),
            wk=_pack_w(w_qkv[:, DIM + col:DIM + col + P]).astype(NPBF16),
            wv=_pack_w(w_qkv[:, 2 * DIM + col:2 * DIM + col + P]).astype(NPBF16),
            bq=(b_qkv[col:col + P] * SCALE).astype(np.float32).reshape(P, 1),
            bk=b_qkv[DIM + col:DIM + col + P].astype(np.float32).reshape(P, 1),
            bv=b_qkv[2 * DIM + col:2 * DIM + col + P].astype(np.float32).reshape(P, 1),
            ct=ct_packed,
            wo=np.ascontiguousarray(w_out[col:col + P, :]).astype(NPBF16),
        )
        for c in range(NC4):
            for g in range(2):
                m[f"xt_{c}_{g}"] = xts[(b, c, g)]
        in_maps.append(m)
    return in_maps, perm, entries, first_e, last_e


_COLMAP = np.array([(q // P % 4) * 512 + (q // P // 4) * P + q % P
                    for q in range(S)])


def unpack_out(arr, perm):
    """[128, 4*2048] bf16 core output -> [2048, 512] f32 in original order."""
    outT = np.zeros((DIM, S), np.float32)
    a = np.asarray(arr, np.float32)
    for oc in range(4):
        outT[oc * P:(oc + 1) * P, :] = a[:, oc * S:(oc + 1) * S]
    out_perm = outT[:, _COLMAP].T  # [2048, 512], rows are permuted queries
    out = np.zeros((S, DIM), np.float32)
    out[perm] = out_perm
    return out


def run(inputs, trace=False, trace_cores=None):
    in_maps, perm, entries, first_e, last_e = make_in_maps(
        inputs["x"], inputs["routes"], inputs["w_qkv"], inputs["b_qkv"],
        inputs["w_out"],
    )
    nc = build_nc(entries, first_e, last_e)
    res = run_bass_kernel_spmd(
        nc, in_maps, list(range(8)), trace=trace, trace_cores=trace_cores,
    )
    b_out = np.asarray(inputs["b_out"], np.float32)
    final = np.zeros((B, S, DIM), np.float32)
    for core in range(8):
        final[core // 4] += unpack_out(res.results[core]["out"], perm)
    final += b_out[None, None, :]
    return final, res


def kernel(**inputs):
    final, _ = run(inputs, trace=False)
    return final
